# revision 11
# baseline (speedup 1.0000x reference)
"""Multi-head causal attention (B=4,T=2048,C=1024,H=16,HS=64) on 8 TRN2 cores.

Sharding: core c -> batch b=c//2, head-group hg=c%2 (8 heads each).
Each core computes QKV projections for its heads, causal flash-attention,
and a partial output projection over its 512 combo channels, emitting
out^T partial [1024, 2048].  Host sums the two partials per batch (the
tensor-parallel all-reduce) and transposes.

All matmuls run in float32r (full PE rate, ~1e-4 rel precision).
Softmax skips max-subtraction (scores ~ N(0,1), exp never overflows);
the denominator comes free as a 65th row of the PV matmul via a
ones-column appended to V.
"""

import sys

if "/opt/trn_rl_repo" not in sys.path:
    sys.path.insert(0, "/opt/trn_rl_repo")

import numpy as np

import concourse.bass as bass
import concourse.mybir as mybir
import concourse.tile as tile
from concourse import bacc
from concourse.bass_utils import run_bass_kernel_spmd

P = 128
B, T, C, H = 4, 2048, 1024, 16
HS = C // H              # 64
HL = H // 2              # 8 local heads per core
HD = HL * HS             # 512 local combo channels
NT = T // 512            # 4 query blocks of 512
NCC = C // P             # 8 contraction chunks over C
NKC = T // P             # 16 key chunks of 128
F32 = mybir.dt.float32
F32R = mybir.dt.float32r
EXP_SCALE = float(HS) ** -0.5  # 1/8, folded into the exp activation

_PROGRAM = None


def _build_program():
    nc = bacc.Bacc("TRN2", target_bir_lowering=False, debug=False, num_devices=8)

    xT = nc.dram_tensor("xT", [C, T], F32, kind="ExternalInput")
    wq = nc.dram_tensor("wq", [C, HD], F32, kind="ExternalInput")
    wk = nc.dram_tensor("wk", [C, HD], F32, kind="ExternalInput")
    wv = nc.dram_tensor("wv", [C, HD], F32, kind="ExternalInput")
    wpT = nc.dram_tensor("wpT", [HD, C], F32, kind="ExternalInput")
    bias = nc.dram_tensor("bias", [C], F32, kind="ExternalInput")
    masks = nc.dram_tensor("masks", [4, P, 512], F32, kind="ExternalInput")
    onescol = nc.dram_tensor("onescol", [P, NKC, HL], F32, kind="ExternalInput")
    outT = nc.dram_tensor("outT", [C, T], F32, kind="ExternalOutput")

    Exp = mybir.ActivationFunctionType.Exp

    with tile.TileContext(nc) as tc:
        with (
            tc.tile_pool(name="persist", bufs=1) as persist,
        ):
            # Q^T / K^T with head pairs stacked on partitions: chunk j holds
            # head 2j in rows 0-63 and head 2j+1 in rows 64-127
            qt = persist.tile([P, HL // 2, T], F32R)
            kt = persist.tile([P, HL // 2, T], F32R)
            vaug = persist.tile([P, NKC, HL, HS + 1], F32R)
            bias_sb = persist.tile([P, C // P], F32)
            nc.sync.dma_start(bias_sb[:], bias[:].rearrange("(db p) -> p db", p=P))
            # ones column for the softmax-denominator row of the PV matmul
            nc.sync.dma_start(
                vaug[:, :, :, HS : HS + 1],
                onescol[:, :, :, None].bitcast(F32R),
            )

            # ---------------- Phase 1: QKV projections ----------------
            with (
                tc.tile_pool(name="wqkv", bufs=1) as wpool,
                tc.tile_pool(name="xt", bufs=2) as xtp,
                tc.tile_pool(name="ps_qk", bufs=2, space="PSUM") as ps_qk,
                tc.tile_pool(name="ps_v", bufs=2, space="PSUM") as ps_v,
            ):
                wq_sb = wpool.tile([P, NCC, HD], F32R, tag="wq")
                wk_sb = wpool.tile([P, NCC, HD], F32R, tag="wk")
                wv_sb = wpool.tile([P, NCC, HD], F32R, tag="wv")
                nc.sync.dma_start(
                    wq_sb[:], wq[:].rearrange("(co p) n -> p co n", p=P).bitcast(F32R)
                )
                nc.sync.dma_start(
                    wk_sb[:], wk[:].rearrange("(co p) n -> p co n", p=P).bitcast(F32R)
                )
                nc.sync.dma_start(
                    wv_sb[:], wv[:].rearrange("(co p) n -> p co n", p=P).bitcast(F32R)
                )

                for tb in range(NT):
                    tsl = slice(tb * 512, (tb + 1) * 512)
                    xt = xtp.tile([P, NCC, 512], F32R, tag="xt")
                    nc.sync.dma_start(
                        xt[:],
                        xT[:].rearrange("(co p) t -> p co t", p=P)[:, :, tsl].bitcast(F32R),
                    )
                    # Q^T and K^T: out [hd-pair 128, t 512]
                    for hb in range(HL // 2):
                        hsl = slice(hb * P, (hb + 1) * P)
                        for w_sb, dst in ((wq_sb, qt), (wk_sb, kt)):
                            pqk = ps_qk.tile([P, 512], F32, tag="pqk")
                            for co in range(NCC):
                                nc.tensor.matmul(
                                    pqk[:],
                                    w_sb[:, co, hsl],
                                    xt[:, co, :],
                                    start=(co == 0),
                                    stop=(co == NCC - 1),
                                )
                            nc.vector.tensor_copy(out=dst[:, hb, tsl], in_=pqk[:])
                    # V: out [t 128, hd 512] -> vaug[:, kc, h, 0:64]
                    for ts2 in range(4):
                        pv = ps_v.tile([P, 512], F32, tag="pv")
                        for co in range(NCC):
                            nc.tensor.matmul(
                                pv[:],
                                xt[:, co, ts2 * P : (ts2 + 1) * P],
                                wv_sb[:, co, :],
                                start=(co == 0),
                                stop=(co == NCC - 1),
                            )
                        kc = tb * 4 + ts2
                        nc.vector.tensor_copy(
                            out=vaug[:, kc, :, 0:HS],
                            in_=pv[:].rearrange("p (h d) -> p h d", h=HL),
                        )

            # ---------------- Phases 2+3: attention + projection ----------------
            with (
                tc.tile_pool(name="attn", bufs=1) as attn,
                tc.tile_pool(name="ptp", bufs=2) as ptp,
                tc.tile_pool(name="misc", bufs=2) as misc,
                tc.tile_pool(name="outp", bufs=3) as outp,
                tc.tile_pool(name="ps_s", bufs=1, space="PSUM") as ps_s,
                tc.tile_pool(name="ps_o", bufs=2, space="PSUM") as ps_o,
                tc.tile_pool(name="ps_p", bufs=2, space="PSUM") as ps_p,
            ):
                comboT = attn.tile([P, HD // P, T], F32R, tag="comboT")
                masks_sb = attn.tile([P, 4, 512], F32R, tag="masks")
                nc.sync.dma_start(
                    masks_sb[:], masks[:].rearrange("i k q -> k i q").bitcast(F32R)
                )
                wpT_sb = attn.tile([P, HD // P, C], F32R, tag="wpT")
                nc.sync.dma_start(
                    wpT_sb[:], wpT[:].rearrange("(co p) n -> p co n", p=P).bitcast(F32R)
                )

                for h in range(HL):
                    r0 = 64 * (h % 2)
                    hp = h // 2
                    for qb in range(NT):
                        q0 = qb * 512
                        qsl = slice(q0, q0 + 512)
                        po = ps_o.tile([P, 512], F32, tag="po")
                        last_kc = qb * 4 + 3
                        for g in range(qb + 1):
                            pss = ps_s.tile([P, 4, 512], F32, tag="pss")
                            for i in range(4):
                                kc = 4 * g + i
                                nc.tensor.matmul(
                                    pss[:, i, :],
                                    kt[r0 : r0 + 64, hp, kc * P : (kc + 1) * P],
                                    qt[r0 : r0 + 64, hp, qsl],
                                    start=True,
                                    stop=True,
                                    tile_position=(r0, 0),
                                )
                            pt = ptp.tile([P, 4, 512], F32R, tag="pt")
                            nc.scalar.activation(pt[:], pss[:], Exp, scale=EXP_SCALE)
                            if g == qb:
                                for i in range(4):
                                    nc.vector.tensor_mul(
                                        out=pt[:, i, :],
                                        in0=pt[:, i, :],
                                        in1=masks_sb[:, i, :],
                                    )
                            for i in range(4):
                                kc = 4 * g + i
                                nc.tensor.matmul(
                                    po[0 : HS + 1, :],
                                    vaug[:, kc, h, :],
                                    pt[:, i, :],
                                    start=(kc == 0),
                                    stop=(kc == last_kc),
                                )
                        rc = misc.tile([1, 512], F32, tag="rc")
                        nc.vector.reciprocal(rc[:], po[HS : HS + 1, :])
                        rb = misc.tile([HS, 512], F32, tag="rb")
                        nc.gpsimd.partition_broadcast(rb[:], rc[:])
                        crow = (h % 2) * 64
                        nc.vector.tensor_mul(
                            out=comboT[crow : crow + 64, h // 2, qsl],
                            in0=po[0:HS, :],
                            in1=rb[:],
                        )

                # projection: out^T [d 128, t 512] blocks
                for tb in range(NT):
                    tsl = slice(tb * 512, (tb + 1) * 512)
                    for db in range(C // P):
                        pp = ps_p.tile([P, 512], F32, tag="pp")
                        for co in range(HD // P):
                            nc.tensor.matmul(
                                pp[:],
                                wpT_sb[:, co, db * P : (db + 1) * P],
                                comboT[:, co, tsl],
                                start=(co == 0),
                                stop=(co == HD // P - 1),
                            )
                        ot = outp.tile([P, 512], F32, tag="ot")
                        nc.vector.tensor_scalar_add(ot[:], pp[:], bias_sb[:, db : db + 1])
                        nc.sync.dma_start(outT[db * P : (db + 1) * P, tsl], ot[:])

    nc.finalize()
    return nc


def _causal_masks():
    # mask_i[kl, ql] = 1.0 iff (128*i + kl) <= ql, for the 4 diagonal key
    # chunks of a 512-wide query block (applied multiplicatively post-exp)
    kl = np.arange(P)[None, :, None]
    ql = np.arange(512)[None, None, :]
    i = np.arange(4)[:, None, None]
    return ((P * i + kl) <= ql).astype(np.float32)


def _in_maps(x, Wq, Wk, Wv, Wproj, bproj):
    masks = _causal_masks()
    zeros_bias = np.zeros_like(bproj)
    onescol = np.ones((P, NKC, HL), dtype=np.float32)
    maps = []
    for core in range(8):
        b, hg = core // 2, core % 2
        hs = slice(hg * HL, (hg + 1) * HL)
        maps.append(
            {
                "xT": np.ascontiguousarray(x[b].T),
                "wq": np.ascontiguousarray(
                    Wq[hs].transpose(1, 0, 2).reshape(C, HD)
                ),
                "wk": np.ascontiguousarray(
                    Wk[hs].transpose(1, 0, 2).reshape(C, HD)
                ),
                "wv": np.ascontiguousarray(
                    Wv[hs].transpose(1, 0, 2).reshape(C, HD)
                ),
                "wpT": np.ascontiguousarray(Wproj[:, hg * HD : (hg + 1) * HD].T),
                "bias": np.ascontiguousarray(bproj if hg == 0 else zeros_bias),
                "masks": masks,
                "onescol": onescol,
            }
        )
    return maps


def get_program():
    global _PROGRAM
    if _PROGRAM is None:
        _PROGRAM = _build_program()
    return _PROGRAM


def kernel(x, Wq, Wk, Wv, Wproj, bproj, _run_kwargs=None):
    x = np.asarray(x, dtype=np.float32)
    Wq = np.asarray(Wq, dtype=np.float32)
    Wk = np.asarray(Wk, dtype=np.float32)
    Wv = np.asarray(Wv, dtype=np.float32)
    Wproj = np.asarray(Wproj, dtype=np.float32)
    bproj = np.asarray(bproj, dtype=np.float32)

    nc = get_program()
    res = run_bass_kernel_spmd(
        nc,
        _in_maps(x, Wq, Wk, Wv, Wproj, bproj),
        core_ids=list(range(8)),
        **(_run_kwargs or {}),
    )
    out = np.empty((B, T, C), dtype=np.float32)
    for b in range(B):
        out[b] = (res.results[2 * b]["outT"] + res.results[2 * b + 1]["outT"]).T
    if _run_kwargs:
        kernel.last_results = res
    return out


# revision 20
# speedup vs baseline: 1.0680x; 1.0680x over previous
"""Multi-head causal attention (B=4,T=2048,C=1024,H=16,HS=64) on 8 TRN2 cores.

Sharding: core c -> batch b=c//2, head-group hg=c%2 (8 heads each).
Each core computes QKV projections for its heads, causal flash-attention,
and a partial output projection over its 512 combo channels, emitting
out^T partial [1024, 2048].  Host sums the two partials per batch (the
tensor-parallel all-reduce) and transposes.

All matmuls run in float32r (full PE rate, ~1e-4 rel precision).
Softmax skips max-subtraction (scores ~ N(0,1), exp never overflows);
the denominator comes free as a 65th row of the PV matmul via a
ones-column appended to V.
"""

import os
import sys

if "/opt/trn_rl_repo" not in sys.path:
    sys.path.insert(0, "/opt/trn_rl_repo")

import ml_dtypes
import numpy as np

import concourse.bass as bass
import concourse.mybir as mybir
import concourse.tile as tile
from concourse import bacc
from concourse.bass_utils import run_bass_kernel_spmd

P = 128
B, T, C, H = 4, 2048, 1024, 16
HS = C // H              # 64
HL = H // 2              # 8 local heads per core
HD = HL * HS             # 512 local combo channels
NT = T // 512            # 4 query blocks of 512
NCC = C // P             # 8 contraction chunks over C
NKC = T // P             # 16 key chunks of 128
F32 = mybir.dt.float32
F32R = mybir.dt.float32r
BF16 = mybir.dt.bfloat16
EXP_SCALE = float(HS) ** -0.5  # 1/8, folded into the exp activation

# matmul input dtype: bf16 (full PE rate @2.4GHz) or f32r (~1.2GHz, 10x tighter)
MM_DT = {"bf16": BF16, "f32r": F32R}[os.environ.get("MM_DT", "bf16")]
MM_NP = {BF16: ml_dtypes.bfloat16, F32R: np.float32}[MM_DT]

_PROGRAM = None


def _build_program(debug_dump=False):
    nc = bacc.Bacc("TRN2", target_bir_lowering=False, debug=False, num_devices=8)
    dbg = {}
    if debug_dump:
        dbg["qt"] = nc.dram_tensor("dbg_qt", [P, HL // 2, T], MM_DT, kind="ExternalOutput")
        dbg["kt"] = nc.dram_tensor("dbg_kt", [P, HL // 2, T], MM_DT, kind="ExternalOutput")
        dbg["vaug"] = nc.dram_tensor("dbg_vaug", [P, NKC, HL, HS + 1], MM_DT, kind="ExternalOutput")
        dbg["comboT"] = nc.dram_tensor("dbg_comboT", [P, HD // P, T], MM_DT, kind="ExternalOutput")
        dbg["pt"] = nc.dram_tensor("dbg_pt", [P, 4, 512], MM_DT, kind="ExternalOutput")
        dbg["po"] = nc.dram_tensor("dbg_po", [P, 512], F32, kind="ExternalOutput")

    xT = nc.dram_tensor("xT", [C, T], MM_DT, kind="ExternalInput")
    wq = nc.dram_tensor("wq", [C, HD], MM_DT, kind="ExternalInput")
    wk = nc.dram_tensor("wk", [C, HD], MM_DT, kind="ExternalInput")
    wv = nc.dram_tensor("wv", [C, HD], MM_DT, kind="ExternalInput")
    wpT = nc.dram_tensor("wpT", [HD, C], MM_DT, kind="ExternalInput")
    bias = nc.dram_tensor("bias", [C], F32, kind="ExternalInput")
    masks = nc.dram_tensor("masks", [4, P, 512], MM_DT, kind="ExternalInput")
    onescol = nc.dram_tensor("onescol", [P, NKC, HL], MM_DT, kind="ExternalInput")
    outT = nc.dram_tensor("outT", [C, T], F32, kind="ExternalOutput")

    Exp = mybir.ActivationFunctionType.Exp

    with tile.TileContext(nc) as tc:
        with (
            tc.tile_pool(name="persist", bufs=1) as persist,
        ):
            # Q^T / K^T with head pairs stacked on partitions: chunk j holds
            # head 2j in rows 0-63 and head 2j+1 in rows 64-127
            qt = persist.tile([P, HL // 2, T], MM_DT)
            kt = persist.tile([P, HL // 2, T], MM_DT)
            vaug = persist.tile([P, NKC, HL, HS + 1], MM_DT)
            bias_sb = persist.tile([P, C // P], F32)
            nc.sync.dma_start(bias_sb[:], bias[:].rearrange("(db p) -> p db", p=P))
            # ones column for the softmax-denominator row of the PV matmul
            nc.sync.dma_start(
                vaug[:, :, :, HS : HS + 1],
                onescol[:, :, :, None],
            )

            # ---------------- Phase 1: QKV projections ----------------
            with (
                tc.tile_pool(name="wqkv", bufs=1) as wpool,
                tc.tile_pool(name="xt", bufs=2) as xtp,
                tc.tile_pool(name="ps_qk", bufs=2, space="PSUM") as ps_qk,
                tc.tile_pool(name="ps_v", bufs=2, space="PSUM") as ps_v,
            ):
                wq_sb = wpool.tile([P, NCC, HD], MM_DT, tag="wq")
                wk_sb = wpool.tile([P, NCC, HD], MM_DT, tag="wk")
                wv_sb = wpool.tile([P, NCC, HD], MM_DT, tag="wv")
                nc.sync.dma_start(
                    wq_sb[:], wq[:].rearrange("(co p) n -> p co n", p=P)
                )
                nc.sync.dma_start(
                    wk_sb[:], wk[:].rearrange("(co p) n -> p co n", p=P)
                )
                nc.sync.dma_start(
                    wv_sb[:], wv[:].rearrange("(co p) n -> p co n", p=P)
                )

                for tb in range(NT):
                    tsl = slice(tb * 512, (tb + 1) * 512)
                    xt = xtp.tile([P, NCC, 512], MM_DT, tag="xt")
                    nc.sync.dma_start(
                        xt[:],
                        xT[:].rearrange("(co p) t -> p co t", p=P)[:, :, tsl],
                    )
                    # Q^T and K^T: out [hd-pair 128, t 512]
                    for hb in range(HL // 2):
                        hsl = slice(hb * P, (hb + 1) * P)
                        for w_sb, dst in ((wq_sb, qt), (wk_sb, kt)):
                            pqk = ps_qk.tile([P, 512], F32, tag="pqk")
                            for co in range(NCC):
                                nc.tensor.matmul(
                                    pqk[:],
                                    w_sb[:, co, hsl],
                                    xt[:, co, :],
                                    start=(co == 0),
                                    stop=(co == NCC - 1),
                                )
                            nc.vector.tensor_copy(out=dst[:, hb, tsl], in_=pqk[:])
                    # V: out [t 128, hd 512] -> vaug[:, kc, h, 0:64]
                    for ts2 in range(4):
                        pv = ps_v.tile([P, 512], F32, tag="pv")
                        for co in range(NCC):
                            nc.tensor.matmul(
                                pv[:],
                                xt[:, co, ts2 * P : (ts2 + 1) * P],
                                wv_sb[:, co, :],
                                start=(co == 0),
                                stop=(co == NCC - 1),
                            )
                        kc = tb * 4 + ts2
                        nc.vector.tensor_copy(
                            out=vaug[:, kc, :, 0:HS],
                            in_=pv[:].rearrange("p (h d) -> p h d", h=HL),
                        )

            # ---------------- Phases 2+3: attention + projection ----------------
            with (
                tc.tile_pool(name="attn", bufs=1) as attn,
                tc.tile_pool(name="ptp", bufs=2) as ptp,
                tc.tile_pool(name="misc", bufs=2) as misc,
                tc.tile_pool(name="outp", bufs=3) as outp,
                tc.tile_pool(name="ps_s", bufs=1, space="PSUM") as ps_s,
                tc.tile_pool(name="ps_o", bufs=2, space="PSUM") as ps_o,
                tc.tile_pool(name="ps_p", bufs=2, space="PSUM") as ps_p,
            ):
                comboT = attn.tile([P, HD // P, T], MM_DT, tag="comboT")
                masks_sb = attn.tile([P, 4, 512], MM_DT, tag="masks")
                nc.sync.dma_start(
                    masks_sb[:], masks[:].rearrange("i k q -> k i q")
                )
                wpT_sb = attn.tile([P, HD // P, C], MM_DT, tag="wpT")
                nc.sync.dma_start(
                    wpT_sb[:], wpT[:].rearrange("(co p) n -> p co n", p=P)
                )

                if debug_dump:
                    nc.sync.dma_start(dbg["qt"][:], qt[:])
                    nc.sync.dma_start(dbg["kt"][:], kt[:])
                    nc.sync.dma_start(dbg["vaug"][:], vaug[:])

                for h in range(HL):
                    r0 = 64 * (h % 2)
                    hp = h // 2
                    for qb in range(NT):
                        q0 = qb * 512
                        qsl = slice(q0, q0 + 512)
                        po = ps_o.tile([P, 512], F32, tag="po")
                        last_kc = qb * 4 + 3
                        for g in range(qb + 1):
                            pss = ps_s.tile([P, 4, 512], F32, tag="pss")
                            for i in range(4):
                                kc = 4 * g + i
                                nc.tensor.matmul(
                                    pss[:, i, :],
                                    kt[r0 : r0 + 64, hp, kc * P : (kc + 1) * P],
                                    qt[r0 : r0 + 64, hp, qsl],
                                    start=True,
                                    stop=True,
                                    tile_position=(r0, 0),
                                )
                            pt = ptp.tile([P, 4, 512], MM_DT, tag="pt")
                            nc.scalar.activation(pt[:], pss[:], Exp, scale=EXP_SCALE)
                            if g == qb:
                                for i in range(4):
                                    nc.vector.tensor_mul(
                                        out=pt[:, i, :],
                                        in0=pt[:, i, :],
                                        in1=masks_sb[:, i, :],
                                    )
                            if debug_dump and h == 0 and qb == 0 and g == 0:
                                nc.sync.dma_start(dbg["pt"][:], pt[:])
                            for i in range(4):
                                kc = 4 * g + i
                                nc.tensor.matmul(
                                    po[0 : HS + 1, :],
                                    vaug[:, kc, h, :],
                                    pt[:, i, :],
                                    start=(kc == 0),
                                    stop=(kc == last_kc),
                                )
                        if debug_dump and h == 0 and qb == 0:
                            dpo = misc.tile([P, 512], F32, tag="dpo")
                            nc.vector.tensor_copy(out=dpo[:], in_=po[:])
                            nc.sync.dma_start(dbg["po"][:], dpo[:])
                        # custom-DVE reciprocal requires partition-0 input:
                        # stage the denominator row via an ACT copy first
                        den = misc.tile([1, 512], F32, tag="den")
                        nc.scalar.copy(out=den[:], in_=po[HS : HS + 1, :])
                        rc = misc.tile([1, 512], F32, tag="rc")
                        nc.vector.reciprocal_approx_fast(rc[:], den[:])
                        rb = misc.tile([HS, 512], F32, tag="rb")
                        nc.gpsimd.partition_broadcast(rb[:], rc[:])
                        crow = (h % 2) * 64
                        nc.vector.tensor_mul(
                            out=comboT[crow : crow + 64, h // 2, qsl],
                            in0=po[0:HS, :],
                            in1=rb[:],
                        )

                if debug_dump:
                    nc.sync.dma_start(dbg["comboT"][:], comboT[:])

                # projection: out^T [d 128, t 512] blocks
                for tb in range(NT):
                    tsl = slice(tb * 512, (tb + 1) * 512)
                    for db in range(C // P):
                        pp = ps_p.tile([P, 512], F32, tag="pp")
                        for co in range(HD // P):
                            nc.tensor.matmul(
                                pp[:],
                                wpT_sb[:, co, db * P : (db + 1) * P],
                                comboT[:, co, tsl],
                                start=(co == 0),
                                stop=(co == HD // P - 1),
                            )
                        ot = outp.tile([P, 512], F32, tag="ot")
                        nc.vector.tensor_scalar_add(ot[:], pp[:], bias_sb[:, db : db + 1])
                        nc.sync.dma_start(outT[db * P : (db + 1) * P, tsl], ot[:])

    nc.finalize()
    return nc


def _causal_masks():
    # mask_i[kl, ql] = 1.0 iff (128*i + kl) <= ql, for the 4 diagonal key
    # chunks of a 512-wide query block (applied multiplicatively post-exp)
    kl = np.arange(P)[None, :, None]
    ql = np.arange(512)[None, None, :]
    i = np.arange(4)[:, None, None]
    return ((P * i + kl) <= ql).astype(np.float32)


def _in_maps(x, Wq, Wk, Wv, Wproj, bproj):
    masks = _causal_masks()
    zeros_bias = np.zeros_like(bproj)
    onescol = np.ones((P, NKC, HL), dtype=MM_NP)
    maps = []
    for core in range(8):
        b, hg = core // 2, core % 2
        hs = slice(hg * HL, (hg + 1) * HL)
        maps.append(
            {
                "xT": np.ascontiguousarray(x[b].T).astype(MM_NP),
                "wq": np.ascontiguousarray(
                    Wq[hs].transpose(1, 0, 2).reshape(C, HD).astype(MM_NP)
                ),
                "wk": np.ascontiguousarray(
                    Wk[hs].transpose(1, 0, 2).reshape(C, HD).astype(MM_NP)
                ),
                "wv": np.ascontiguousarray(
                    Wv[hs].transpose(1, 0, 2).reshape(C, HD).astype(MM_NP)
                ),
                "wpT": np.ascontiguousarray(Wproj[:, hg * HD : (hg + 1) * HD].T).astype(MM_NP),
                "bias": np.ascontiguousarray(bproj if hg == 0 else zeros_bias),
                "masks": masks.astype(MM_NP),
                "onescol": onescol,
            }
        )
    return maps


def get_program():
    global _PROGRAM
    if _PROGRAM is None:
        _PROGRAM = _build_program()
    return _PROGRAM


def kernel(x, Wq, Wk, Wv, Wproj, bproj, _run_kwargs=None):
    x = np.asarray(x, dtype=np.float32)
    Wq = np.asarray(Wq, dtype=np.float32)
    Wk = np.asarray(Wk, dtype=np.float32)
    Wv = np.asarray(Wv, dtype=np.float32)
    Wproj = np.asarray(Wproj, dtype=np.float32)
    bproj = np.asarray(bproj, dtype=np.float32)

    nc = get_program()
    res = run_bass_kernel_spmd(
        nc,
        _in_maps(x, Wq, Wk, Wv, Wproj, bproj),
        core_ids=list(range(8)),
        **(_run_kwargs or {}),
    )
    out = np.empty((B, T, C), dtype=np.float32)
    for b in range(B):
        out[b] = (res.results[2 * b]["outT"] + res.results[2 * b + 1]["outT"]).T
    if _run_kwargs:
        kernel.last_results = res
    return out


# revision 23
# speedup vs baseline: 1.1042x; 1.0339x over previous
"""Multi-head causal attention (B=4,T=2048,C=1024,H=16,HS=64) on 8 TRN2 cores.

Sharding: core c -> batch b=c//2, head-group hg=c%2 (8 heads each).
Each core computes QKV projections for its heads, causal flash-attention,
and a partial output projection over its 512 combo channels, emitting
out^T partial [1024, 2048].  Host sums the two partials per batch (the
tensor-parallel all-reduce) and transposes.

All matmuls run in float32r (full PE rate, ~1e-4 rel precision).
Softmax skips max-subtraction (scores ~ N(0,1), exp never overflows);
the denominator comes free as a 65th row of the PV matmul via a
ones-column appended to V.
"""

import os
import sys

if "/opt/trn_rl_repo" not in sys.path:
    sys.path.insert(0, "/opt/trn_rl_repo")

import ml_dtypes
import numpy as np

import concourse.bass as bass
import concourse.mybir as mybir
import concourse.tile as tile
from concourse import bacc
from concourse.bass_utils import run_bass_kernel_spmd

P = 128
B, T, C, H = 4, 2048, 1024, 16
HS = C // H              # 64
HL = H // 2              # 8 local heads per core
HD = HL * HS             # 512 local combo channels
NT = T // 512            # 4 query blocks of 512
NCC = C // P             # 8 contraction chunks over C
NKC = T // P             # 16 key chunks of 128
F32 = mybir.dt.float32
F32R = mybir.dt.float32r
BF16 = mybir.dt.bfloat16
EXP_SCALE = float(HS) ** -0.5  # 1/8, folded into the exp activation

# matmul input dtype: bf16 (full PE rate @2.4GHz) or f32r (~1.2GHz, 10x tighter)
MM_DT = {"bf16": BF16, "f32r": F32R}[os.environ.get("MM_DT", "bf16")]
MM_NP = {BF16: ml_dtypes.bfloat16, F32R: np.float32}[MM_DT]

_PROGRAM = None


def _build_program(debug_dump=False):
    nc = bacc.Bacc("TRN2", target_bir_lowering=False, debug=False, num_devices=8)
    dbg = {}
    if debug_dump:
        dbg["qt"] = nc.dram_tensor("dbg_qt", [P, HL // 2, T], MM_DT, kind="ExternalOutput")
        dbg["kt"] = nc.dram_tensor("dbg_kt", [P, HL // 2, T], MM_DT, kind="ExternalOutput")
        dbg["vaug"] = nc.dram_tensor("dbg_vaug", [P, NKC, HL, HS + 1], MM_DT, kind="ExternalOutput")
        dbg["comboT"] = nc.dram_tensor("dbg_comboT", [P, HD // P, T], MM_DT, kind="ExternalOutput")
        dbg["pt"] = nc.dram_tensor("dbg_pt", [P, 4, 512], MM_DT, kind="ExternalOutput")
        dbg["po"] = nc.dram_tensor("dbg_po", [P, 512], F32, kind="ExternalOutput")

    xT = nc.dram_tensor("xT", [C, T], MM_DT, kind="ExternalInput")
    wq = nc.dram_tensor("wq", [C, HD], MM_DT, kind="ExternalInput")
    wk = nc.dram_tensor("wk", [C, HD], MM_DT, kind="ExternalInput")
    wv = nc.dram_tensor("wv", [C, HD], MM_DT, kind="ExternalInput")
    wpT = nc.dram_tensor("wpT", [HD, C], MM_DT, kind="ExternalInput")
    bias = nc.dram_tensor("bias", [C], F32, kind="ExternalInput")
    masks = nc.dram_tensor("masks", [4, P, 512], MM_DT, kind="ExternalInput")
    onescol = nc.dram_tensor("onescol", [P, NKC, HL], MM_DT, kind="ExternalInput")
    outT = nc.dram_tensor("outT", [C, T], F32, kind="ExternalOutput")

    Exp = mybir.ActivationFunctionType.Exp

    with tile.TileContext(nc) as tc:
        with (
            tc.tile_pool(name="persist", bufs=1) as persist,
        ):
            # Q^T / K^T with head pairs stacked on partitions: chunk j holds
            # head 2j in rows 0-63 and head 2j+1 in rows 64-127
            qt = persist.tile([P, HL // 2, T], MM_DT)
            kt = persist.tile([P, HL // 2, T], MM_DT)
            vaug = persist.tile([P, NKC, HL, HS + 1], MM_DT)
            bias_sb = persist.tile([P, C // P], F32)
            nc.sync.dma_start(bias_sb[:], bias[:].rearrange("(db p) -> p db", p=P))
            # ones column for the softmax-denominator row of the PV matmul
            nc.sync.dma_start(
                vaug[:, :, :, HS : HS + 1],
                onescol[:, :, :, None],
            )

            # ---------------- Phase 1: QKV projections ----------------
            with (
                tc.tile_pool(name="wqkv", bufs=1) as wpool,
                tc.tile_pool(name="xt", bufs=2) as xtp,
                tc.tile_pool(name="ps_qk", bufs=2, space="PSUM") as ps_qk,
                tc.tile_pool(name="ps_v", bufs=2, space="PSUM") as ps_v,
            ):
                wq_sb = wpool.tile([P, NCC, HD], MM_DT, tag="wq")
                wk_sb = wpool.tile([P, NCC, HD], MM_DT, tag="wk")
                wv_sb = wpool.tile([P, NCC, HD], MM_DT, tag="wv")
                nc.sync.dma_start(
                    wq_sb[:], wq[:].rearrange("(co p) n -> p co n", p=P)
                )
                nc.sync.dma_start(
                    wk_sb[:], wk[:].rearrange("(co p) n -> p co n", p=P)
                )
                nc.sync.dma_start(
                    wv_sb[:], wv[:].rearrange("(co p) n -> p co n", p=P)
                )

                for tb in range(NT):
                    tsl = slice(tb * 512, (tb + 1) * 512)
                    xt = xtp.tile([P, NCC, 512], MM_DT, tag="xt")
                    nc.sync.dma_start(
                        xt[:],
                        xT[:].rearrange("(co p) t -> p co t", p=P)[:, :, tsl],
                    )
                    # Q^T and K^T: out [hd-pair 128, t 512]
                    for hb in range(HL // 2):
                        hsl = slice(hb * P, (hb + 1) * P)
                        for w_sb, dst in ((wq_sb, qt), (wk_sb, kt)):
                            pqk = ps_qk.tile([P, 512], F32, tag="pqk")
                            for co in range(NCC):
                                nc.tensor.matmul(
                                    pqk[:],
                                    w_sb[:, co, hsl],
                                    xt[:, co, :],
                                    start=(co == 0),
                                    stop=(co == NCC - 1),
                                )
                            nc.vector.tensor_copy(out=dst[:, hb, tsl], in_=pqk[:])
                    # V: out [t 128, hd 512] -> vaug[:, kc, h, 0:64]
                    for ts2 in range(4):
                        pv = ps_v.tile([P, 512], F32, tag="pv")
                        for co in range(NCC):
                            nc.tensor.matmul(
                                pv[:],
                                xt[:, co, ts2 * P : (ts2 + 1) * P],
                                wv_sb[:, co, :],
                                start=(co == 0),
                                stop=(co == NCC - 1),
                            )
                        kc = tb * 4 + ts2
                        nc.vector.tensor_copy(
                            out=vaug[:, kc, :, 0:HS],
                            in_=pv[:].rearrange("p (h d) -> p h d", h=HL),
                        )

            # ---------------- Phases 2+3: attention + projection ----------------
            with (
                tc.tile_pool(name="attn", bufs=1) as attn,
                tc.tile_pool(name="ptp", bufs=2) as ptp,
                tc.tile_pool(name="misc", bufs=2) as misc,
                tc.tile_pool(name="outp", bufs=3) as outp,
                tc.tile_pool(name="ps_s", bufs=1, space="PSUM") as ps_s,
                tc.tile_pool(name="ps_o", bufs=2, space="PSUM") as ps_o,
                tc.tile_pool(name="ps_p", bufs=2, space="PSUM") as ps_p,
            ):
                comboT = attn.tile([P, HD // P, T], MM_DT, tag="comboT")
                masks_sb = attn.tile([P, 4, 512], MM_DT, tag="masks")
                nc.sync.dma_start(
                    masks_sb[:], masks[:].rearrange("i k q -> k i q")
                )
                wpT_sb = attn.tile([P, HD // P, C], MM_DT, tag="wpT")
                nc.sync.dma_start(
                    wpT_sb[:], wpT[:].rearrange("(co p) n -> p co n", p=P)
                )

                if debug_dump:
                    nc.sync.dma_start(dbg["qt"][:], qt[:])
                    nc.sync.dma_start(dbg["kt"][:], kt[:])
                    nc.sync.dma_start(dbg["vaug"][:], vaug[:])

                # one persistent 4-bank scores psum: diagonal-suffix matmuls
                # intentionally leave stale (bounded) data in masked columns,
                # which same-tensor reuse keeps visible to the dep tracker
                pss = ps_s.tile([P, 4, 512], F32, tag="pss")
                for h in range(HL):
                    r0 = 64 * (h % 2)
                    hp = h // 2
                    for qb in range(NT):
                        q0 = qb * 512
                        qsl = slice(q0, q0 + 512)
                        po = ps_o.tile([P, 512], F32, tag="po")
                        last_kc = qb * 4 + 3
                        for g in range(qb + 1):
                            for i in range(4):
                                kc = 4 * g + i
                                # diagonal tiles: columns q < kc*128-q0 are
                                # fully masked; skip computing them (the psum
                                # there holds bounded stale scores from the
                                # previous group — exp'd then zeroed by the
                                # mask).  Only when qb>0 so first-ever use of
                                # the psum bank is always a full-width write.
                                c0 = kc * P - q0 if (g == qb and qb > 0) else 0
                                nc.tensor.matmul(
                                    pss[:, i, c0:512],
                                    kt[r0 : r0 + 64, hp, kc * P : (kc + 1) * P],
                                    qt[r0 : r0 + 64, hp, q0 + c0 : q0 + 512],
                                    start=True,
                                    stop=True,
                                    tile_position=(r0, 0),
                                )
                            pt = ptp.tile([P, 4, 512], MM_DT, tag="pt")
                            nc.scalar.activation(pt[:], pss[:], Exp, scale=EXP_SCALE)
                            if g == qb:
                                for i in range(4):
                                    nc.vector.tensor_mul(
                                        out=pt[:, i, :],
                                        in0=pt[:, i, :],
                                        in1=masks_sb[:, i, :],
                                    )
                            if debug_dump and h == 0 and qb == 0 and g == 0:
                                nc.sync.dma_start(dbg["pt"][:], pt[:])
                            for i in range(4):
                                kc = 4 * g + i
                                # diagonal tiles: PT columns q < kc*128-q0 are
                                # zero (masked) — skip accumulating them
                                c0 = max(0, kc * P - q0) if g == qb else 0
                                nc.tensor.matmul(
                                    po[0 : HS + 1, c0:512],
                                    vaug[:, kc, h, :],
                                    pt[:, i, c0:512],
                                    start=(kc == 0),
                                    stop=(kc == last_kc),
                                )
                        if debug_dump and h == 0 and qb == 0:
                            dpo = misc.tile([P, 512], F32, tag="dpo")
                            nc.vector.tensor_copy(out=dpo[:], in_=po[:])
                            nc.sync.dma_start(dbg["po"][:], dpo[:])
                        # custom-DVE reciprocal requires partition-0 input:
                        # stage the denominator row via an ACT copy first
                        den = misc.tile([1, 512], F32, tag="den")
                        nc.scalar.copy(out=den[:], in_=po[HS : HS + 1, :])
                        rc = misc.tile([1, 512], F32, tag="rc")
                        nc.vector.reciprocal_approx_fast(rc[:], den[:])
                        rb = misc.tile([HS, 512], F32, tag="rb")
                        nc.gpsimd.partition_broadcast(rb[:], rc[:])
                        crow = (h % 2) * 64
                        nc.vector.tensor_mul(
                            out=comboT[crow : crow + 64, h // 2, qsl],
                            in0=po[0:HS, :],
                            in1=rb[:],
                        )

                if debug_dump:
                    nc.sync.dma_start(dbg["comboT"][:], comboT[:])

                # projection: out^T [d 128, t 512] blocks
                for tb in range(NT):
                    tsl = slice(tb * 512, (tb + 1) * 512)
                    for db in range(C // P):
                        pp = ps_p.tile([P, 512], F32, tag="pp")
                        for co in range(HD // P):
                            nc.tensor.matmul(
                                pp[:],
                                wpT_sb[:, co, db * P : (db + 1) * P],
                                comboT[:, co, tsl],
                                start=(co == 0),
                                stop=(co == HD // P - 1),
                            )
                        ot = outp.tile([P, 512], F32, tag="ot")
                        nc.vector.tensor_scalar_add(ot[:], pp[:], bias_sb[:, db : db + 1])
                        nc.sync.dma_start(outT[db * P : (db + 1) * P, tsl], ot[:])

    nc.finalize()
    return nc


def _causal_masks():
    # mask_i[kl, ql] = 1.0 iff (128*i + kl) <= ql, for the 4 diagonal key
    # chunks of a 512-wide query block (applied multiplicatively post-exp)
    kl = np.arange(P)[None, :, None]
    ql = np.arange(512)[None, None, :]
    i = np.arange(4)[:, None, None]
    return ((P * i + kl) <= ql).astype(np.float32)


def _in_maps(x, Wq, Wk, Wv, Wproj, bproj):
    masks = _causal_masks()
    zeros_bias = np.zeros_like(bproj)
    onescol = np.ones((P, NKC, HL), dtype=MM_NP)
    maps = []
    for core in range(8):
        b, hg = core // 2, core % 2
        hs = slice(hg * HL, (hg + 1) * HL)
        maps.append(
            {
                "xT": np.ascontiguousarray(x[b].T).astype(MM_NP),
                "wq": np.ascontiguousarray(
                    Wq[hs].transpose(1, 0, 2).reshape(C, HD).astype(MM_NP)
                ),
                "wk": np.ascontiguousarray(
                    Wk[hs].transpose(1, 0, 2).reshape(C, HD).astype(MM_NP)
                ),
                "wv": np.ascontiguousarray(
                    Wv[hs].transpose(1, 0, 2).reshape(C, HD).astype(MM_NP)
                ),
                "wpT": np.ascontiguousarray(Wproj[:, hg * HD : (hg + 1) * HD].T).astype(MM_NP),
                "bias": np.ascontiguousarray(bproj if hg == 0 else zeros_bias),
                "masks": masks.astype(MM_NP),
                "onescol": onescol,
            }
        )
    return maps


def get_program():
    global _PROGRAM
    if _PROGRAM is None:
        _PROGRAM = _build_program()
    return _PROGRAM


def kernel(x, Wq, Wk, Wv, Wproj, bproj, _run_kwargs=None):
    x = np.asarray(x, dtype=np.float32)
    Wq = np.asarray(Wq, dtype=np.float32)
    Wk = np.asarray(Wk, dtype=np.float32)
    Wv = np.asarray(Wv, dtype=np.float32)
    Wproj = np.asarray(Wproj, dtype=np.float32)
    bproj = np.asarray(bproj, dtype=np.float32)

    nc = get_program()
    res = run_bass_kernel_spmd(
        nc,
        _in_maps(x, Wq, Wk, Wv, Wproj, bproj),
        core_ids=list(range(8)),
        **(_run_kwargs or {}),
    )
    out = np.empty((B, T, C), dtype=np.float32)
    for b in range(B):
        out[b] = (res.results[2 * b]["outT"] + res.results[2 * b + 1]["outT"]).T
    if _run_kwargs:
        kernel.last_results = res
    return out
